# revision 22
# baseline (speedup 1.0000x reference)
"""CurricularFace loss kernel for 8 Trainium2 NeuronCores.

Strategy: tensor-parallel over out_features (classes). Each core owns a
12800-column shard of `kernel` (100000 padded to 102400) and computes its
[C_shard, N] slice of the S-scaled logits in transposed layout.

Everything that depends only on small inputs is precomputed on the host in
fp64 (column inverse-norms, the per-row target path: target logit, t update,
cos_theta_m thresholds, final target logits) and shipped as small tiles.
The device runs a pure partial-FC pipeline per 512-column block:

  raw   = k_c . e_n                          (PE, fp32r, PSUM)
  r1    = relu(S + raw * (-S/||k_c||))       (ACT, per-partition scale)
  cneg  = min(r1, 2S) - S   = -S*clip(u)     (DVE tensor_scalar, 2x mode)
  h     = (cneg/sqrt(S) + bh)^2 = z + bh^2   (ACT, z = S*(c^2 + (t-1)c))
  m     = r1 < thr_n                         (Pool tensor_tensor)
  g     = (h - s0) * m                       (DVE stt, s0 = bh^2)
  o     = -cneg + g = S*where(m, c*(t+c), c) (DVE/Pool split, bf16 out)

The target-column scatter is applied on the host.
"""
import math

import numpy as np

import concourse.bass as bass
import concourse.bacc as bacc
import concourse.mybir as mybir
import concourse.tile as tile
from concourse.bass_utils import run_bass_kernel_spmd

fp32 = mybir.dt.float32
fp32r = mybir.dt.float32r
bf16 = mybir.dt.bfloat16
i16 = mybir.dt.int16
ALU = mybir.AluOpType
ACTF = mybir.ActivationFunctionType

MARGIN = 0.5
S = 64.0
SQS = math.sqrt(S)
COS_M = math.cos(MARGIN)
SIN_M = math.sin(MARGIN)
THRESHOLD = math.cos(math.pi - MARGIN)
MM = math.sin(math.pi - MARGIN) * MARGIN
EPS = 1e-12

N = 512          # batch rows
D = 512          # in_features
C = 100000       # classes
NCORES = 8
CS = 12800       # per-core (padded) column shard
NBLK = CS // 512  # 25 blocks of 512 columns

_prog_cache = {}


def _build_program():
    nc = bacc.Bacc(None, target_bir_lowering=False)

    embT_d = nc.dram_tensor("embT", [D, N], fp32r, kind="ExternalInput")
    ksh_d = nc.dram_tensor("ksh", [D, CS], fp32r, kind="ExternalInput")
    nsv_d = nc.dram_tensor("nsv", [128, NBLK * 4], fp32, kind="ExternalInput")
    thr_d = nc.dram_tensor("thr", [128, 4, N], i16, kind="ExternalInput")
    bh_d = nc.dram_tensor("bh", [128, 1], fp32, kind="ExternalInput")
    s0_d = nc.dram_tensor("s0", [128, 1], fp32, kind="ExternalInput")
    outT_d = nc.dram_tensor("outT", [CS, N], bf16, kind="ExternalOutput")

    with tile.TileContext(nc) as tc:
        with (
            tc.tile_pool(name="const", bufs=1) as cp,
            tc.tile_pool(name="kin", bufs=2) as kin,
            tc.tile_pool(name="work", bufs=3) as wk,
            tc.tile_pool(name="psraw", bufs=8, space="PSUM") as psraw,
        ):
            # ---- persistent tiles (host-precomputed) ----
            emb_sb = cp.tile([128, 4, N], fp32r, tag="emb")
            nsv = cp.tile([128, NBLK * 4], fp32, tag="nsv")
            thr_b4 = cp.tile([128, 4, N], i16, tag="thr_b4")
            bh128 = cp.tile([128, 1], fp32, tag="bh128")
            s0128 = cp.tile([128, 1], fp32, tag="s0128")
            biasS = cp.tile([128, 1], fp32, tag="biasS")

            nc.sync.dma_start(nsv[:], nsv_d[:])
            nc.sync.dma_start(bh128[:], bh_d[:])
            nc.sync.dma_start(s0128[:], s0_d[:])
            embr = embT_d[:].rearrange("(g p) n -> p g n", p=128)
            for g in range(4):
                nc.sync.dma_start(emb_sb[:, g, :], embr[:, g, :])
            nc.vector.memset(biasS[:], 128.0 * S)

            # ---- main loop over 25 column blocks, double-buffered loads ----
            def load_kblk(blk, split=False):
                kblk = kin.tile([128, 4, 512], fp32r, tag="kblk")
                src = ksh_d[:, bass.ts(blk, 512)].rearrange("(g p) c -> p g c", p=128)
                if split:  # j-sliced so the first matmuls start sooner
                    for j in range(4):
                        nc.sync.dma_start(
                            kblk[:, :, bass.ts(j, 128)], src[:, :, bass.ts(j, 128)]
                        )
                else:
                    nc.sync.dma_start(kblk[:], src)
                return kblk

            def head(blk, kblk):
                """PE matmuls + ACT r1 + DVE cneg for block blk."""
                r1 = wk.tile([128, 4, N], i16, tag="r1")
                for j in range(4):
                    raw = psraw.tile([128, N], fp32, tag="raw")
                    for g in range(4):
                        nc.tensor.matmul(
                            raw[:],
                            kblk[:, g, bass.ts(j, 128)],
                            emb_sb[:, g, :],
                            start=(g == 0),
                            stop=(g == 3),
                        )
                    nc.scalar.activation(
                        r1[:, j, :], raw[:], ACTF.Relu,
                        bias=biasS[:], scale=nsv[:, blk * 4 + j : blk * 4 + j + 1],
                    )
                cpos = wk.tile([128, 4, N], bf16, tag="cneg")
                nc.vector.tensor_scalar(
                    cpos[:], r1[:], 128.0 * 2.0 * S, 1.0 / 128.0, ALU.min, ALU.mult
                )
                m = wk.tile([128, 4, N], bf16, tag="m")
                nc.vector.tensor_tensor(m[:], r1[:], thr_b4[:], ALU.is_lt)
                return r1, cpos, m

            def tail(blk, r1, cpos, m):
                """ACT h + DVE g + Pool o + store for block blk."""
                h = wk.tile([128, 4, N], fp32, tag="h")
                nc.scalar.activation(
                    h[:], r1[:], ACTF.Square, bias=bh128[:], scale=1.0 / (128.0 * SQS)
                )
                g_t = wk.tile([128, 4, N], bf16, tag="g_t")
                nc.vector.scalar_tensor_tensor(
                    g_t[:], h[:], s0128[:], m[:], ALU.subtract, ALU.mult
                )
                o_t = wk.tile([128, 4, N], bf16, tag="o_t")
                # o = g - cpos = out - S (host re-adds S); Pool supports tt-sub
                nc.gpsimd.tensor_tensor(o_t[:], g_t[:], cpos[:], ALU.subtract)
                nc.sync.dma_start(
                    outT_d[bass.ts(blk, 512), :].rearrange("(j p) n -> p j n", p=128),
                    o_t[:],
                )

            def tail_fine(blk, r1, cpos, m):
                """Last-block tail: j-split chains to shorten the drain."""
                h = wk.tile([128, 4, N], fp32, tag="h")
                g_t = wk.tile([128, 4, N], bf16, tag="g_t")
                o_t = wk.tile([128, 4, N], bf16, tag="o_t")
                for j in range(4):
                    nc.scalar.activation(
                        h[:, j, :], r1[:, j, :], ACTF.Square,
                        bias=bh128[:], scale=1.0 / (128.0 * SQS),
                    )
                    nc.vector.scalar_tensor_tensor(
                        g_t[:, j, :], h[:, j, :], s0128[:], m[:, j, :],
                        ALU.subtract, ALU.mult,
                    )
                    eng = nc.gpsimd if j % 2 == 0 else nc.vector
                    eng.tensor_tensor(
                        o_t[:, j, :], g_t[:, j, :], cpos[:, j, :], ALU.subtract
                    )
                    nc.sync.dma_start(
                        outT_d[bass.ts(blk, 512), :].rearrange(
                            "(j p) n -> p j n", p=128
                        )[:, j, :],
                        o_t[:, j, :],
                    )

            # software pipeline: tail(i-1) is emitted after head(i)
            kblk_cur = load_kblk(0, split=True)
            nc.sync.dma_start(thr_b4[:], thr_d[:])  # after kblk0: not on critical path
            prev = None
            for blk in range(NBLK):
                hcm = head(blk, kblk_cur)
                if blk + 1 < NBLK:
                    kblk_cur = load_kblk(blk + 1)
                if prev is not None:
                    tail(blk - 1, *prev)
                prev = hcm
            tail_fine(NBLK - 1, *prev)

    nc.finalize()
    return nc


def _get_program():
    if "nc" not in _prog_cache:
        _prog_cache["nc"] = _build_program()
    return _prog_cache["nc"]


def prepare(embeddings, label, kernel, t):
    """Host-side prep: per-core input maps + host scatter values."""
    embeddings = np.asarray(embeddings, dtype=np.float32)
    label = np.asarray(label).astype(np.int64)
    kern = np.asarray(kernel, dtype=np.float32)
    t = np.asarray(t, dtype=np.float32)

    embT = np.ascontiguousarray(embeddings.T)                      # [D, N]

    # column inverse norms (fp64 host)
    n2 = np.einsum("dc,dc->c", kern, kern, dtype=np.float64)       # [C]
    norms = np.sqrt(n2)
    inv = 1.0 / np.maximum(norms, EPS)                             # [C]

    # target path (fp64 host)
    gk = kern[:, label].astype(np.float64)                         # [D, N]
    tlraw = np.einsum("dn,dn->n", embT.astype(np.float64), gk)     # [N]
    tl = np.clip(tlraw * inv[label], -1.0, 1.0)
    sin = np.sqrt(np.maximum(1.0 - tl * tl, 0.0))
    ctm = tl * COS_M - sin * SIN_M
    t_new = float(tl.mean() * 0.01 + 0.99 * float(t.reshape(-1)[0]))
    ftl = np.where(tl > THRESHOLD, ctm, tl - MM) * S               # [N] scatter vals
    thr = (S * (1.0 - ctm)).astype(np.float32)                     # [N]
    thri = np.round(128.0 * thr).astype(np.int16)                  # fixed-point
    thr_b4 = np.ascontiguousarray(np.broadcast_to(thri[None, None, :], (128, 4, N)))
    bh = -(t_new + 1.0) * SQS / 2.0
    s0 = S * (t_new - 1.0) ** 2 / 4.0
    bh128 = np.full((128, 1), bh, dtype=np.float32)
    s0128 = np.full((128, 1), s0, dtype=np.float32)

    # padded inverse norms, per-core [128, NBLK*4] tiles: nsv[p, blk*4+j] =
    # -S/||k_c||, c = core*CS + blk*512 + j*128 + p
    inv_pad = np.full(NCORES * CS, 1.0, dtype=np.float64)
    inv_pad[:C] = inv
    nsv_all = (-128.0 * S * inv_pad).astype(np.float32)

    in_maps = []
    for i in range(NCORES):
        lo, hi = i * CS, (i + 1) * CS
        if hi <= C:
            ksh = np.ascontiguousarray(kern[:, lo:hi])
        else:
            pad = np.ones((D, hi - C), dtype=np.float32)
            ksh = np.ascontiguousarray(np.concatenate([kern[:, lo:C], pad], axis=1))
        nsv_i = np.ascontiguousarray(
            nsv_all[lo:hi].reshape(NBLK, 4, 128).transpose(2, 0, 1).reshape(128, NBLK * 4)
        )
        in_maps.append({
            "embT": embT, "ksh": ksh, "nsv": nsv_i,
            "thr": thr_b4, "bh": bh128, "s0": s0128,
        })
    return in_maps, ftl.astype(np.float32), label


def kernel(embeddings, label, kernel, t):
    in_maps, ftl, label64 = prepare(embeddings, label, kernel, t)

    nc = _get_program()
    res = run_bass_kernel_spmd(nc, in_maps, core_ids=list(range(NCORES)))
    _prog_cache["last_res"] = res

    outT = np.concatenate(
        [np.add(np.asarray(r["outT"]), np.float32(S), dtype=np.float32)
         for r in res.results], axis=0
    )[:C]                                                          # [C, N]
    out = np.ascontiguousarray(outT.T)                             # [N, C]
    out[np.arange(N), label64] = ftl
    return out
